# revision 13
# baseline (speedup 1.0000x reference)
"""Attention block (single head) on 8 TRN2 NeuronCores.

Reference (per batch element b):
    Q = x @ Wq; K = x @ Wk; V = x @ Wv          (x: [S, D], W*: [D, D])
    out = softmax(Q @ K^T / sqrt(D)) @ V

Sharding: data-parallel over batch B=8 -> one batch element per core.
No collectives needed; weights are replicated.

v2 algebraic restructure: scores = Q K^T = x (Wq Wk^T) x^T. The host
computes M = Wq @ Wk^T in fp32 (tiny, 512^3) and ships M instead of
Wq/Wk. On-core this kills the whole K projection (1/3 of phase-1 PE
time): only G^T = M^T x^T and V = x Wv are projected, and the scores
matmul contracts G against x^T slices already resident in SBUF.

DMA structure: HBM DMA here is descriptor-bound (~155ns per partition-
row descriptor regardless of 1-4KB line size; the 16 SDMA engines drain
all rings as one FIFO stream, ~410 GB/s at 4KB lines), so inputs ship
in host-prearranged layouts where each SBUF partition row is one
contiguous 4KB DRAM line (128 descriptors per transfer — splitting any
transfer only adds descriptors and slows the critical path). Transfers
are enqueued in consumption order with the first-matmul dependency set
(wm + x-chunk-0, 1MB = HBM-roofline ~2.5us) leading two different DGE
rings. Output: one DMA per q-tile (2KB lines) on the sync (HWDGE) ring,
whose sequencer is idle in phase 2; the last q-tile splits h-halves
across sync+scalar so the tail-critical DMA starts as early as possible.

All matmul operands are bf16; accumulation stays fp32 in PSUM and the
output is written fp32. End-to-end rel err vs the fp32 reference is
~3.8e-3 (tolerance 2e-2).

Per-core layout (S=2048, D=512, P=128):
  xt_all [128, 4, 4, 512]: x^T as [p, chunk, dtile, s'], DMA'd directly
      from the host-pre-transposed x. Reused as the scores lhsT
      (contraction runs over the d' features of M).
  GT[di] [128, 2048] = G^T  (lhsT=M slice, rhs=xT).
  V_full[si] [128, 2, 258]: V in two 256-halves, a ones column at free
      index 256 of each half (softmax denominator), col 257 zero padding.
  S^T [k, q] chunks = x @ G^T  (lhsT=xT k-slice, rhs=GT 512-chunk).
  E^T = exp(S^T / sqrt(D))     (ScalarE, PSUM -> SBUF, bf16 out).
  AV:  psum[q-tile, 258|256] = sum_k E^T-slice @ [V half | 1 | 0]; h=0's
      col 256 is rowsum(E); normalize via DVE reciprocal + tensor_scalar
      mul into a [128, 512] staging tile.

HAM warmup: the PE clock-gate starts at 1.2 GHz and only reaches 2.4 GHz
after ~3.4us of sustained matmul activity. Dummy N=128 matmuls on a
memset tile run during the input DMA wait to flip the clock gate early.
"""

import contextlib

import ml_dtypes
import numpy as np

from concourse import bacc, mybir, tile
from concourse.bass_utils import run_bass_kernel_spmd

P = 128
S = 2048
D = 512
B = 8
N_CORES = 8
SCALE = float(1.0 / np.sqrt(D))

F32 = mybir.dt.float32
BF16 = mybir.dt.bfloat16

N_ST = S // P    # 16 s-tiles (also k-tiles)
N_DT = D // P    # 4 d-tiles (input dim, also d'-tiles)
N_QC = S // 512  # 4 q-chunks of 512

WARMUP_MM = 48   # dummy N=128 matmuls to flip the HAM clock gate early


def _xt(xt_all, di, s0, s1):
    """Slice x^T [128, d-block di, global s range) out of the chunked layout."""
    c0, o0 = divmod(s0, 512)
    c1, o1 = divmod(s1 - 1, 512)
    assert c0 == c1, (s0, s1)
    return xt_all[:, c0, di, o0:o1 + 1]


def _emit(nc, tc, x, wm, wv, out):
    ctx = contextlib.ExitStack()
    with ctx:
        wpool = ctx.enter_context(tc.tile_pool(name="wpool", bufs=1))
        persist = ctx.enter_context(tc.tile_pool(name="persist", bufs=1))
        misc = ctx.enter_context(tc.tile_pool(name="misc", bufs=2))
        xtp = ctx.enter_context(tc.tile_pool(name="xt", bufs=1))
        etp = ctx.enter_context(tc.tile_pool(name="et", bufs=1))
        ost = ctx.enter_context(tc.tile_pool(name="ostage", bufs=2))
        ps = ctx.enter_context(tc.tile_pool(name="ps", bufs=1, space="PSUM"))

        ones2 = misc.tile([P, 2, 2], BF16, tag="ones2")
        nc.vector.memset(ones2[:, :, :], 0.0)
        nc.vector.memset(ones2[:, :, 0:1], 1.0)

        # PE warmup: dummy matmuls on a memset tile while input DMA is in
        # flight; they rotate through the real psum buffers.
        warm_sb = misc.tile([P, P], BF16, tag="warm")
        nc.vector.memset(warm_sb[:, :], 0.0)
        for i in range(WARMUP_MM):
            pw = ps.tile([P, 512], F32, tag="mm512", bufs=5, name=f"warm{i}")
            nc.tensor.matmul(pw[:, 0:P], warm_sb[:, :], warm_sb[:, :],
                             start=True, stop=True)

        # Inputs arrive in host-prearranged layouts: every partition row is
        # one contiguous 4KB DRAM line (descriptor-efficient).
        wm_t = wpool.tile([P, N_DT, D], BF16, tag="wm", name="wm")
        wv_t = wpool.tile([P, N_DT, D], BF16, tag="wv", name="wv")
        xt_all = xtp.tile([P, N_QC, N_DT, 512], BF16, tag="xt_all")

        nc.sync.dma_start(
            wm_t[:, :, :], wm.rearrange("p (a e) -> p a e", a=N_DT)
        )
        nc.scalar.dma_start(
            xt_all[:, 0, :, :], x[0].rearrange("p (a s) -> p a s", a=N_DT)
        )
        nc.gpsimd.dma_start(
            wv_t[:, :, :], wv.rearrange("p (a e) -> p a e", a=N_DT)
        )
        nc.sync.dma_start(
            xt_all[:, 1, :, :], x[1].rearrange("p (a s) -> p a s", a=N_DT)
        )
        nc.scalar.dma_start(
            xt_all[:, 2, :, :], x[2].rearrange("p (a s) -> p a s", a=N_DT)
        )
        nc.sync.dma_start(
            xt_all[:, 3, :, :], x[3].rearrange("p (a s) -> p a s", a=N_DT)
        )
        wv_sb = [wv_t[:, di, :] for di in range(N_DT)]

        gt_sb = [persist.tile([P, S], BF16, tag=f"gt{di}", name=f"gt{di}") for di in range(N_DT)]
        v_sb = [persist.tile([P, 2, 258], BF16, tag=f"v{si}", name=f"v{si}") for si in range(N_ST)]

        for si in range(N_ST):
            nc.vector.tensor_copy(v_sb[si][:, :, 256:258], ones2[:, :, :])

        # ---------- phase 1: project G^T and V from pre-transposed x ----------
        for sc in range(N_QC):
            cs = slice(sc * 512, (sc + 1) * 512)

            for ei in range(N_DT):
                pg = ps.tile([P, 512], F32, tag="mm512", bufs=5, name=f"pg{sc}_{ei}")
                for di in range(N_DT):
                    nc.tensor.matmul(
                        pg[:, :], wm_t[:, di, ei * P:(ei + 1) * P], xt_all[:, sc, di, :],
                        start=(di == 0), stop=(di == N_DT - 1),
                    )
                nc.scalar.copy(gt_sb[ei][:, cs], pg[:, :])

            for si in range(sc * 4, sc * 4 + 4):
                pv = ps.tile([P, D], F32, tag="mm512", bufs=5, name=f"pv{si}")
                for di in range(N_DT):
                    nc.tensor.matmul(
                        pv[:, :], _xt(xt_all, di, si * P, (si + 1) * P), wv_sb[di][:, :],
                        start=(di == 0), stop=(di == N_DT - 1),
                    )
                nc.scalar.copy(
                    v_sb[si][:, :, 0:256],
                    pv[:, :].rearrange("p (a b) -> p a b", a=2),
                )

        # ---------- phase 2: scores, softmax, AV ----------
        # et tiles are double-buffered by q-chunk and the emission order
        # is scores(0), scores(1), AV(0), scores(2), AV(1), ... so
        # ScalarE computes the next chunk's exps while the PE runs the
        # previous chunk's AV chains.
        def emit_scores(qc):
            qs_all = slice(qc * 512, (qc + 1) * 512)
            et_sb = []
            for ki in range(N_ST):
                pst = ps.tile([P, 512], F32, tag="mm512", bufs=5, name=f"pst{qc}_{ki}")
                for di in range(N_DT):
                    nc.tensor.matmul(
                        pst[:, :], _xt(xt_all, di, ki * P, (ki + 1) * P), gt_sb[di][:, qs_all],
                        start=(di == 0), stop=(di == N_DT - 1),
                    )
                et = etp.tile(
                    [P, 512], BF16, tag=f"et{qc % 2}_{ki}", name=f"et{qc}_{ki}"
                )
                nc.scalar.activation(
                    et[:, :], pst[:, :],
                    mybir.ActivationFunctionType.Exp, scale=SCALE,
                )
                et_sb.append(et)
            return et_sb

        def emit_av(qc, et_sb):
            for qs in range(4):
                qi = qc * 4 + qs
                o_tile = ost.tile([P, D], F32, tag="o", name=f"o{qi}")
                r_sb = misc.tile([P, 1], F32, tag="r", name=f"r{qi}")
                for h in range(2):
                    w_av = 258 if h == 0 else 256
                    pav = ps.tile([P, w_av], F32, tag="tpav", bufs=3, name=f"pav{qi}_{h}")
                    for ki in range(N_ST):
                        nc.tensor.matmul(
                            pav[:, :],
                            et_sb[ki][:, qs * P:(qs + 1) * P],
                            v_sb[ki][:, h, 0:w_av],
                            start=(ki == 0), stop=(ki == N_ST - 1),
                        )
                    if h == 0:
                        nc.vector.reciprocal(r_sb[:, :], pav[:, 256:257])
                    nc.vector.tensor_scalar_mul(
                        o_tile[:, h * 256:(h + 1) * 256],
                        pav[:, 0:256],
                        r_sb[:, :],
                    )
                    if qi == S // P - 1:
                        # tail-critical: ship each half as soon as its
                        # normalize lands, on its own HWDGE ring
                        (nc.sync, nc.scalar)[h].dma_start(
                            out[qi * P:(qi + 1) * P, h * 256:(h + 1) * 256],
                            o_tile[:, h * 256:(h + 1) * 256],
                        )
                if qi != S // P - 1:
                    nc.sync.dma_start(
                        out[qi * P:(qi + 1) * P, :], o_tile[:, :]
                    )

        prev = emit_scores(0)
        for qc in range(1, N_QC):
            cur = emit_scores(qc)
            emit_av(qc - 1, prev)
            prev = cur
        emit_av(N_QC - 1, prev)


_CACHED_NC = None


def _build():
    global _CACHED_NC
    if _CACHED_NC is not None:
        return _CACHED_NC
    nc = bacc.Bacc(
        "TRN2", target_bir_lowering=False, debug=False, num_devices=N_CORES
    )
    x = nc.declare_dram_parameter("x", [N_QC, P, N_DT * 512], BF16, isOutput=False)
    wm = nc.declare_dram_parameter("wm", [P, N_DT * D], BF16, isOutput=False)
    wv = nc.declare_dram_parameter("wv", [P, N_DT * D], BF16, isOutput=False)
    out = nc.declare_dram_parameter("out", [S, D], F32, isOutput=True)
    with tile.TileContext(nc) as tc:
        _emit(nc, tc, x.ap(), wm.ap(), wv.ap(), out.ap())
    nc.compile()
    _CACHED_NC = nc
    return nc


def _in_maps(x, Wq, Wk, Wv):
    bf = ml_dtypes.bfloat16
    # x [B, S, D] -> x^T [D, S] -> [c, p, a*s'] chunk-major, 4KB lines
    xt = np.asarray(x, dtype=np.float32).transpose(0, 2, 1)  # [B, D, S]
    xh = np.ascontiguousarray(
        xt.reshape(B, N_DT, P, N_QC, 512).transpose(0, 3, 2, 1, 4)
        .reshape(B, N_QC, P, N_DT * 512)
    ).astype(bf)
    M = np.asarray(Wq, dtype=np.float32) @ np.asarray(Wk, dtype=np.float32).T
    wm = np.ascontiguousarray(
        M.reshape(N_DT, P, D).transpose(1, 0, 2).reshape(P, N_DT * D)
    ).astype(bf)
    # wv d-major: [p, a, e]
    wv = np.ascontiguousarray(
        np.asarray(Wv, dtype=np.float32).reshape(N_DT, P, D).transpose(1, 0, 2)
        .reshape(P, N_DT * D)
    ).astype(bf)
    return [
        {"x": xh[b], "wm": wm, "wv": wv} for b in range(B)
    ]


def kernel(x, Wq, Wk, Wv, **_ignored):
    nc = _build()
    in_maps = _in_maps(x, Wq, Wk, Wv)
    res = run_bass_kernel_spmd(
        nc, in_maps, core_ids=list(range(N_CORES)), trace=False
    )
    return np.stack([res.results[b]["out"] for b in range(B)], axis=0)


# revision 14
# speedup vs baseline: 1.0040x; 1.0040x over previous
"""Attention block (single head) on 8 TRN2 NeuronCores.

Reference (per batch element b):
    Q = x @ Wq; K = x @ Wk; V = x @ Wv          (x: [S, D], W*: [D, D])
    out = softmax(Q @ K^T / sqrt(D)) @ V

Sharding: data-parallel over batch B=8 -> one batch element per core.
No collectives needed; weights are replicated.

v2 algebraic restructure: scores = Q K^T = x (Wq Wk^T) x^T. The host
computes M = Wq @ Wk^T in fp32 (tiny, 512^3) and ships M instead of
Wq/Wk. On-core this kills the whole K projection (1/3 of phase-1 PE
time): only G^T = M^T x^T and V = x Wv are projected, and the scores
matmul contracts G against x^T slices already resident in SBUF.

DMA structure: HBM DMA here is descriptor-bound (~155ns per partition-
row descriptor regardless of 1-4KB line size; the 16 SDMA engines drain
all rings as one FIFO stream, ~410 GB/s at 4KB lines), so inputs ship
in host-prearranged layouts where each SBUF partition row is one
contiguous 4KB DRAM line (128 descriptors per transfer — splitting any
transfer only adds descriptors and slows the critical path). Transfers
are enqueued in consumption order with the first-matmul dependency set
(wm + x-chunk-0, 1MB = HBM-roofline ~2.5us) leading two different DGE
rings. Output: one DMA per q-tile (2KB lines) on the sync (HWDGE) ring,
whose sequencer is idle in phase 2; the last q-tile splits h-halves
across sync+scalar so the tail-critical DMA starts as early as possible.

All matmul operands are bf16; accumulation stays fp32 in PSUM and the
output is written fp32. End-to-end rel err vs the fp32 reference is
~3.8e-3 (tolerance 2e-2).

Per-core layout (S=2048, D=512, P=128):
  xt_all [128, 4, 4, 512]: x^T as [p, chunk, dtile, s'], DMA'd directly
      from the host-pre-transposed x. Reused as the scores lhsT
      (contraction runs over the d' features of M).
  GT[di] [128, 2048] = G^T  (lhsT=M slice, rhs=xT).
  V_full[si] [128, 2, 258]: V in two 256-halves, a ones column at free
      index 256 of each half (softmax denominator), col 257 zero padding.
  S^T [k, q] chunks = x @ G^T  (lhsT=xT k-slice, rhs=GT 512-chunk).
  E^T = exp(S^T / sqrt(D))     (ScalarE, PSUM -> SBUF, bf16 out).
  AV:  psum[q-tile, 258|256] = sum_k E^T-slice @ [V half | 1 | 0]; h=0's
      col 256 is rowsum(E); normalize via DVE reciprocal + tensor_scalar
      mul into a [128, 512] staging tile.

HAM warmup: the PE clock-gate starts at 1.2 GHz and only reaches 2.4 GHz
after ~3.4us of sustained matmul activity. Dummy N=128 matmuls on a
memset tile run during the input DMA wait to flip the clock gate early.
"""

import contextlib

import ml_dtypes
import numpy as np

from concourse import bacc, mybir, tile
from concourse.bass_utils import run_bass_kernel_spmd

P = 128
S = 2048
D = 512
B = 8
N_CORES = 8
SCALE = float(1.0 / np.sqrt(D))

F32 = mybir.dt.float32
BF16 = mybir.dt.bfloat16

N_ST = S // P    # 16 s-tiles (also k-tiles)
N_DT = D // P    # 4 d-tiles (input dim, also d'-tiles)
N_QC = S // 512  # 4 q-chunks of 512

WARMUP_MM = 48   # dummy N=128 matmuls to flip the HAM clock gate early


def _xt(xt_all, di, s0, s1):
    """Slice x^T [128, d-block di, global s range) out of the chunked layout."""
    c0, o0 = divmod(s0, 512)
    c1, o1 = divmod(s1 - 1, 512)
    assert c0 == c1, (s0, s1)
    return xt_all[:, c0, di, o0:o1 + 1]


def _emit(nc, tc, x, wm, wv, out):
    ctx = contextlib.ExitStack()
    with ctx:
        wpool = ctx.enter_context(tc.tile_pool(name="wpool", bufs=1))
        persist = ctx.enter_context(tc.tile_pool(name="persist", bufs=1))
        misc = ctx.enter_context(tc.tile_pool(name="misc", bufs=2))
        xtp = ctx.enter_context(tc.tile_pool(name="xt", bufs=1))
        etp = ctx.enter_context(tc.tile_pool(name="et", bufs=1))
        ost = ctx.enter_context(tc.tile_pool(name="ostage", bufs=2))
        ps = ctx.enter_context(tc.tile_pool(name="ps", bufs=1, space="PSUM"))

        ones2 = misc.tile([P, 2, 2], BF16, tag="ones2")
        nc.vector.memset(ones2[:, :, :], 0.0)
        nc.vector.memset(ones2[:, :, 0:1], 1.0)

        # PE warmup: dummy matmuls on a memset tile while input DMA is in
        # flight; they rotate through the real psum buffers.
        warm_sb = misc.tile([P, P], BF16, tag="warm")
        nc.vector.memset(warm_sb[:, :], 0.0)
        for i in range(WARMUP_MM):
            pw = ps.tile([P, 512], F32, tag="mm512", bufs=5, name=f"warm{i}")
            nc.tensor.matmul(pw[:, 0:P], warm_sb[:, :], warm_sb[:, :],
                             start=True, stop=True)

        # Inputs arrive in host-prearranged layouts: every partition row is
        # one contiguous 4KB DRAM line (descriptor-efficient).
        wm_t = wpool.tile([P, N_DT, D], BF16, tag="wm", name="wm")
        wv_t = wpool.tile([P, N_DT, D], BF16, tag="wv", name="wv")
        xt_all = xtp.tile([P, N_QC, N_DT, 512], BF16, tag="xt_all")

        nc.sync.dma_start(
            wm_t[:, :, :], wm.rearrange("p (a e) -> p a e", a=N_DT)
        )
        nc.scalar.dma_start(
            xt_all[:, 0, :, :], x[0].rearrange("p (a s) -> p a s", a=N_DT)
        )
        # wv rides sync right behind wm: on the slow-starting SWDGE
        # (gpsimd) ring it would drain after c1/c2 in the global FIFO and
        # stall the first V-projection by ~0.7us
        nc.sync.dma_start(
            wv_t[:, :, :], wv.rearrange("p (a e) -> p a e", a=N_DT)
        )
        nc.scalar.dma_start(
            xt_all[:, 1, :, :], x[1].rearrange("p (a s) -> p a s", a=N_DT)
        )
        nc.sync.dma_start(
            xt_all[:, 2, :, :], x[2].rearrange("p (a s) -> p a s", a=N_DT)
        )
        nc.scalar.dma_start(
            xt_all[:, 3, :, :], x[3].rearrange("p (a s) -> p a s", a=N_DT)
        )
        wv_sb = [wv_t[:, di, :] for di in range(N_DT)]

        gt_sb = [persist.tile([P, S], BF16, tag=f"gt{di}", name=f"gt{di}") for di in range(N_DT)]
        v_sb = [persist.tile([P, 2, 258], BF16, tag=f"v{si}", name=f"v{si}") for si in range(N_ST)]

        for si in range(N_ST):
            nc.vector.tensor_copy(v_sb[si][:, :, 256:258], ones2[:, :, :])

        # ---------- phase 1: project G^T and V from pre-transposed x ----------
        for sc in range(N_QC):
            cs = slice(sc * 512, (sc + 1) * 512)

            for ei in range(N_DT):
                pg = ps.tile([P, 512], F32, tag="mm512", bufs=5, name=f"pg{sc}_{ei}")
                for di in range(N_DT):
                    nc.tensor.matmul(
                        pg[:, :], wm_t[:, di, ei * P:(ei + 1) * P], xt_all[:, sc, di, :],
                        start=(di == 0), stop=(di == N_DT - 1),
                    )
                nc.scalar.copy(gt_sb[ei][:, cs], pg[:, :])

            for si in range(sc * 4, sc * 4 + 4):
                pv = ps.tile([P, D], F32, tag="mm512", bufs=5, name=f"pv{si}")
                for di in range(N_DT):
                    nc.tensor.matmul(
                        pv[:, :], _xt(xt_all, di, si * P, (si + 1) * P), wv_sb[di][:, :],
                        start=(di == 0), stop=(di == N_DT - 1),
                    )
                nc.scalar.copy(
                    v_sb[si][:, :, 0:256],
                    pv[:, :].rearrange("p (a b) -> p a b", a=2),
                )

        # ---------- phase 2: scores, softmax, AV ----------
        # et tiles are double-buffered by q-chunk and the emission order
        # is scores(0), scores(1), AV(0), scores(2), AV(1), ... so
        # ScalarE computes the next chunk's exps while the PE runs the
        # previous chunk's AV chains.
        def emit_scores(qc):
            qs_all = slice(qc * 512, (qc + 1) * 512)
            et_sb = []
            for ki in range(N_ST):
                pst = ps.tile([P, 512], F32, tag="mm512", bufs=5, name=f"pst{qc}_{ki}")
                for di in range(N_DT):
                    nc.tensor.matmul(
                        pst[:, :], _xt(xt_all, di, ki * P, (ki + 1) * P), gt_sb[di][:, qs_all],
                        start=(di == 0), stop=(di == N_DT - 1),
                    )
                et = etp.tile(
                    [P, 512], BF16, tag=f"et{qc % 2}_{ki}", name=f"et{qc}_{ki}"
                )
                nc.scalar.activation(
                    et[:, :], pst[:, :],
                    mybir.ActivationFunctionType.Exp, scale=SCALE,
                )
                et_sb.append(et)
            return et_sb

        def emit_av(qc, et_sb):
            for qs in range(4):
                qi = qc * 4 + qs
                o_tile = ost.tile([P, D], F32, tag="o", name=f"o{qi}")
                r_sb = misc.tile([P, 1], F32, tag="r", name=f"r{qi}")
                for h in range(2):
                    w_av = 258 if h == 0 else 256
                    pav = ps.tile([P, w_av], F32, tag="tpav", bufs=3, name=f"pav{qi}_{h}")
                    for ki in range(N_ST):
                        nc.tensor.matmul(
                            pav[:, :],
                            et_sb[ki][:, qs * P:(qs + 1) * P],
                            v_sb[ki][:, h, 0:w_av],
                            start=(ki == 0), stop=(ki == N_ST - 1),
                        )
                    if h == 0:
                        nc.vector.reciprocal(r_sb[:, :], pav[:, 256:257])
                    nc.vector.tensor_scalar_mul(
                        o_tile[:, h * 256:(h + 1) * 256],
                        pav[:, 0:256],
                        r_sb[:, :],
                    )
                    if qi == S // P - 1:
                        # tail-critical: ship each half as soon as its
                        # normalize lands, on its own HWDGE ring
                        (nc.sync, nc.scalar)[h].dma_start(
                            out[qi * P:(qi + 1) * P, h * 256:(h + 1) * 256],
                            o_tile[:, h * 256:(h + 1) * 256],
                        )
                if qi != S // P - 1:
                    nc.sync.dma_start(
                        out[qi * P:(qi + 1) * P, :], o_tile[:, :]
                    )

        prev = emit_scores(0)
        for qc in range(1, N_QC):
            cur = emit_scores(qc)
            emit_av(qc - 1, prev)
            prev = cur
        emit_av(N_QC - 1, prev)


_CACHED_NC = None


def _build():
    global _CACHED_NC
    if _CACHED_NC is not None:
        return _CACHED_NC
    nc = bacc.Bacc(
        "TRN2", target_bir_lowering=False, debug=False, num_devices=N_CORES
    )
    x = nc.declare_dram_parameter("x", [N_QC, P, N_DT * 512], BF16, isOutput=False)
    wm = nc.declare_dram_parameter("wm", [P, N_DT * D], BF16, isOutput=False)
    wv = nc.declare_dram_parameter("wv", [P, N_DT * D], BF16, isOutput=False)
    out = nc.declare_dram_parameter("out", [S, D], F32, isOutput=True)
    with tile.TileContext(nc) as tc:
        _emit(nc, tc, x.ap(), wm.ap(), wv.ap(), out.ap())
    nc.compile()
    _CACHED_NC = nc
    return nc


def _in_maps(x, Wq, Wk, Wv):
    bf = ml_dtypes.bfloat16
    # x [B, S, D] -> x^T [D, S] -> [c, p, a*s'] chunk-major, 4KB lines
    xt = np.asarray(x, dtype=np.float32).transpose(0, 2, 1)  # [B, D, S]
    xh = np.ascontiguousarray(
        xt.reshape(B, N_DT, P, N_QC, 512).transpose(0, 3, 2, 1, 4)
        .reshape(B, N_QC, P, N_DT * 512)
    ).astype(bf)
    M = np.asarray(Wq, dtype=np.float32) @ np.asarray(Wk, dtype=np.float32).T
    wm = np.ascontiguousarray(
        M.reshape(N_DT, P, D).transpose(1, 0, 2).reshape(P, N_DT * D)
    ).astype(bf)
    # wv d-major: [p, a, e]
    wv = np.ascontiguousarray(
        np.asarray(Wv, dtype=np.float32).reshape(N_DT, P, D).transpose(1, 0, 2)
        .reshape(P, N_DT * D)
    ).astype(bf)
    return [
        {"x": xh[b], "wm": wm, "wv": wv} for b in range(B)
    ]


def kernel(x, Wq, Wk, Wv, **_ignored):
    nc = _build()
    in_maps = _in_maps(x, Wq, Wk, Wv)
    res = run_bass_kernel_spmd(
        nc, in_maps, core_ids=list(range(N_CORES)), trace=False
    )
    return np.stack([res.results[b]["out"] for b in range(B)], axis=0)


# revision 15
# speedup vs baseline: 1.0068x; 1.0028x over previous
"""Attention block (single head) on 8 TRN2 NeuronCores.

Reference (per batch element b):
    Q = x @ Wq; K = x @ Wk; V = x @ Wv          (x: [S, D], W*: [D, D])
    out = softmax(Q @ K^T / sqrt(D)) @ V

Sharding: data-parallel over batch B=8 -> one batch element per core.
No collectives needed; weights are replicated.

v2 algebraic restructure: scores = Q K^T = x (Wq Wk^T) x^T. The host
computes M = Wq @ Wk^T in fp32 (tiny, 512^3) and ships M instead of
Wq/Wk. On-core this kills the whole K projection (1/3 of phase-1 PE
time): only G^T = M^T x^T and V = x Wv are projected, and the scores
matmul contracts G against x^T slices already resident in SBUF.

DMA structure: HBM DMA here is descriptor-bound (~155ns per partition-
row descriptor regardless of 1-4KB line size; the 16 SDMA engines drain
all rings as one FIFO stream, ~410 GB/s at 4KB lines), so inputs ship
in host-prearranged layouts where each SBUF partition row is one
contiguous 4KB DRAM line (128 descriptors per transfer — splitting any
transfer only adds descriptors and slows the critical path). Transfers
are enqueued in consumption order with the first-matmul dependency set
(wm + x-chunk-0, 1MB = HBM-roofline ~2.5us) leading two different DGE
rings. Output: one DMA per q-tile (2KB lines) on the sync (HWDGE) ring,
whose sequencer is idle in phase 2; the last q-tile splits h-halves
across sync+scalar so the tail-critical DMA starts as early as possible.

All matmul operands are bf16; accumulation stays fp32 in PSUM and the
output is written fp32. End-to-end rel err vs the fp32 reference is
~3.8e-3 (tolerance 2e-2).

Per-core layout (S=2048, D=512, P=128):
  xt_all [128, 4, 4, 512]: x^T as [p, chunk, dtile, s'], DMA'd directly
      from the host-pre-transposed x. Reused as the scores lhsT
      (contraction runs over the d' features of M).
  GT[di] [128, 2048] = G^T  (lhsT=M slice, rhs=xT).
  V_full[si] [128, 2, 258]: V in two 256-halves, a ones column at free
      index 256 of each half (softmax denominator), col 257 zero padding.
  S^T [k, q] chunks = x @ G^T  (lhsT=xT k-slice, rhs=GT 512-chunk).
  E^T = exp(S^T / sqrt(D))     (ScalarE, PSUM -> SBUF, bf16 out).
  AV:  psum[q-tile, 258|256] = sum_k E^T-slice @ [V half | 1 | 0]; h=0's
      col 256 is rowsum(E); normalize via DVE reciprocal + tensor_scalar
      mul into a [128, 512] staging tile.

HAM warmup: the PE clock-gate starts at 1.2 GHz and only reaches 2.4 GHz
after ~3.4us of sustained matmul activity. Dummy N=128 matmuls on a
memset tile run during the input DMA wait to flip the clock gate early.
"""

import contextlib

import ml_dtypes
import numpy as np

from concourse import bacc, mybir, tile
from concourse.bass_utils import run_bass_kernel_spmd

P = 128
S = 2048
D = 512
B = 8
N_CORES = 8
SCALE = float(1.0 / np.sqrt(D))

F32 = mybir.dt.float32
BF16 = mybir.dt.bfloat16

N_ST = S // P    # 16 s-tiles (also k-tiles)
N_DT = D // P    # 4 d-tiles (input dim, also d'-tiles)
N_QC = S // 512  # 4 q-chunks of 512

WARMUP_MM = 48   # dummy N=128 matmuls to flip the HAM clock gate early


def _xt(xt_all, di, s0, s1):
    """Slice x^T [128, d-block di, global s range) out of the chunked layout."""
    c0, o0 = divmod(s0, 512)
    c1, o1 = divmod(s1 - 1, 512)
    assert c0 == c1, (s0, s1)
    return xt_all[:, c0, di, o0:o1 + 1]


def _emit(nc, tc, x, wm, wv, out):
    ctx = contextlib.ExitStack()
    with ctx:
        wpool = ctx.enter_context(tc.tile_pool(name="wpool", bufs=1))
        persist = ctx.enter_context(tc.tile_pool(name="persist", bufs=1))
        misc = ctx.enter_context(tc.tile_pool(name="misc", bufs=2))
        xtp = ctx.enter_context(tc.tile_pool(name="xt", bufs=1))
        etp = ctx.enter_context(tc.tile_pool(name="et", bufs=1))
        ost = ctx.enter_context(tc.tile_pool(name="ostage", bufs=2))
        ps = ctx.enter_context(tc.tile_pool(name="ps", bufs=1, space="PSUM"))

        ones2 = misc.tile([P, 2, 2], BF16, tag="ones2")
        nc.vector.memset(ones2[:, :, :], 0.0)
        nc.vector.memset(ones2[:, :, 0:1], 1.0)

        # PE warmup: dummy matmuls on a memset tile while input DMA is in
        # flight; they rotate through the real psum buffers.
        warm_sb = misc.tile([P, P], BF16, tag="warm")
        nc.vector.memset(warm_sb[:, :], 0.0)
        for i in range(WARMUP_MM):
            pw = ps.tile([P, 512], F32, tag="mm512", bufs=5, name=f"warm{i}")
            nc.tensor.matmul(pw[:, 0:P], warm_sb[:, :], warm_sb[:, :],
                             start=True, stop=True)

        # Inputs arrive in host-prearranged layouts: every partition row is
        # one contiguous 4KB DRAM line (descriptor-efficient).
        wm_t = wpool.tile([P, N_DT, D], BF16, tag="wm", name="wm")
        wv_t = wpool.tile([P, N_DT, D], BF16, tag="wv", name="wv")
        xt_all = xtp.tile([P, N_QC, N_DT, 512], BF16, tag="xt_all")

        nc.sync.dma_start(
            wm_t[:, :, :], wm.rearrange("p (a e) -> p a e", a=N_DT)
        )
        nc.scalar.dma_start(
            xt_all[:, 0, :, :], x[0].rearrange("p (a s) -> p a s", a=N_DT)
        )
        # wv rides sync right behind wm: on the slow-starting SWDGE
        # (gpsimd) ring it would drain after c1/c2 in the global FIFO and
        # stall the first V-projection by ~0.7us
        nc.sync.dma_start(
            wv_t[:, :, :], wv.rearrange("p (a e) -> p a e", a=N_DT)
        )
        nc.scalar.dma_start(
            xt_all[:, 1, :, :], x[1].rearrange("p (a s) -> p a s", a=N_DT)
        )
        nc.sync.dma_start(
            xt_all[:, 2, :, :], x[2].rearrange("p (a s) -> p a s", a=N_DT)
        )
        nc.scalar.dma_start(
            xt_all[:, 3, :, :], x[3].rearrange("p (a s) -> p a s", a=N_DT)
        )
        wv_sb = [wv_t[:, di, :] for di in range(N_DT)]

        gt_sb = [persist.tile([P, S], BF16, tag=f"gt{di}", name=f"gt{di}") for di in range(N_DT)]
        v_sb = [persist.tile([P, 2, 258], BF16, tag=f"v{si}", name=f"v{si}") for si in range(N_ST)]

        for si in range(N_ST):
            nc.vector.tensor_copy(v_sb[si][:, :, 256:258], ones2[:, :, :])

        # ---------- phase 1: project G^T and V from pre-transposed x ----------
        for sc in range(N_QC):
            cs = slice(sc * 512, (sc + 1) * 512)

            for ei in range(N_DT):
                pg = ps.tile([P, 512], F32, tag="mm512", bufs=5, name=f"pg{sc}_{ei}")
                for di in range(N_DT):
                    nc.tensor.matmul(
                        pg[:, :], wm_t[:, di, ei * P:(ei + 1) * P], xt_all[:, sc, di, :],
                        start=(di == 0), stop=(di == N_DT - 1),
                    )
                nc.scalar.copy(gt_sb[ei][:, cs], pg[:, :])

            for si in range(sc * 4, sc * 4 + 4):
                pv = ps.tile([P, D], F32, tag="mm512", bufs=5, name=f"pv{si}")
                for di in range(N_DT):
                    nc.tensor.matmul(
                        pv[:, :], _xt(xt_all, di, si * P, (si + 1) * P), wv_sb[di][:, :],
                        start=(di == 0), stop=(di == N_DT - 1),
                    )
                nc.scalar.copy(
                    v_sb[si][:, :, 0:256],
                    pv[:, :].rearrange("p (a b) -> p a b", a=2),
                )

        # ---------- phase 2: scores, softmax, AV ----------
        # et tiles are double-buffered by q-chunk and the emission order
        # is scores(0), scores(1), AV(0), scores(2), AV(1), ... so
        # ScalarE computes the next chunk's exps while the PE runs the
        # previous chunk's AV chains.
        def emit_scores(qc):
            qs_all = slice(qc * 512, (qc + 1) * 512)
            et_sb = []
            for ki in range(N_ST):
                pst = ps.tile([P, 512], F32, tag="mm512", bufs=5, name=f"pst{qc}_{ki}")
                for di in range(N_DT):
                    nc.tensor.matmul(
                        pst[:, :], _xt(xt_all, di, ki * P, (ki + 1) * P), gt_sb[di][:, qs_all],
                        start=(di == 0), stop=(di == N_DT - 1),
                    )
                et = etp.tile(
                    [P, 512], BF16, tag=f"et{qc % 2}_{ki}", name=f"et{qc}_{ki}"
                )
                nc.scalar.activation(
                    et[:, :], pst[:, :],
                    mybir.ActivationFunctionType.Exp, scale=SCALE,
                )
                et_sb.append(et)
            return et_sb

        def emit_av(qc, et_sb):
            for qs in range(4):
                qi = qc * 4 + qs
                o_tile = ost.tile([P, D], F32, tag="o", name=f"o{qi}")
                r_sb = misc.tile([P, 1], F32, tag="r", name=f"r{qi}")
                for h in range(2):
                    w_av = 258 if h == 0 else 256
                    pav = ps.tile([P, w_av], F32, tag="tpav", bufs=3, name=f"pav{qi}_{h}")
                    for ki in range(N_ST):
                        nc.tensor.matmul(
                            pav[:, :],
                            et_sb[ki][:, qs * P:(qs + 1) * P],
                            v_sb[ki][:, h, 0:w_av],
                            start=(ki == 0), stop=(ki == N_ST - 1),
                        )
                    if h == 0:
                        nc.vector.reciprocal(r_sb[:, :], pav[:, 256:257])
                    nc.vector.tensor_scalar_mul(
                        o_tile[:, h * 256:(h + 1) * 256],
                        pav[:, 0:256],
                        r_sb[:, :],
                    )
                    if qi == S // P - 1:
                        # tail-critical: ship each half as soon as its
                        # normalize lands, on its own HWDGE ring
                        (nc.sync, nc.scalar)[h].dma_start(
                            out[qi * P:(qi + 1) * P, h * 256:(h + 1) * 256],
                            o_tile[:, h * 256:(h + 1) * 256],
                        )
                if qi != S // P - 1:
                    nc.sync.dma_start(
                        out[qi * P:(qi + 1) * P, :], o_tile[:, :]
                    )

        prev = emit_scores(0)
        for qc in range(1, N_QC):
            cur = emit_scores(qc)
            emit_av(qc - 1, prev)
            prev = cur
        emit_av(N_QC - 1, prev)


_CACHED_NC = None


def _build():
    global _CACHED_NC
    if _CACHED_NC is not None:
        return _CACHED_NC
    nc = bacc.Bacc(
        "TRN2", target_bir_lowering=False, debug=False, num_devices=1
    )
    x = nc.declare_dram_parameter("x", [N_QC, P, N_DT * 512], BF16, isOutput=False)
    wm = nc.declare_dram_parameter("wm", [P, N_DT * D], BF16, isOutput=False)
    wv = nc.declare_dram_parameter("wv", [P, N_DT * D], BF16, isOutput=False)
    out = nc.declare_dram_parameter("out", [S, D], F32, isOutput=True)
    with tile.TileContext(nc) as tc:
        _emit(nc, tc, x.ap(), wm.ap(), wv.ap(), out.ap())
    nc.compile()
    _CACHED_NC = nc
    return nc


def _in_maps(x, Wq, Wk, Wv):
    bf = ml_dtypes.bfloat16
    # x [B, S, D] -> x^T [D, S] -> [c, p, a*s'] chunk-major, 4KB lines
    xt = np.asarray(x, dtype=np.float32).transpose(0, 2, 1)  # [B, D, S]
    xh = np.ascontiguousarray(
        xt.reshape(B, N_DT, P, N_QC, 512).transpose(0, 3, 2, 1, 4)
        .reshape(B, N_QC, P, N_DT * 512)
    ).astype(bf)
    M = np.asarray(Wq, dtype=np.float32) @ np.asarray(Wk, dtype=np.float32).T
    wm = np.ascontiguousarray(
        M.reshape(N_DT, P, D).transpose(1, 0, 2).reshape(P, N_DT * D)
    ).astype(bf)
    # wv d-major: [p, a, e]
    wv = np.ascontiguousarray(
        np.asarray(Wv, dtype=np.float32).reshape(N_DT, P, D).transpose(1, 0, 2)
        .reshape(P, N_DT * D)
    ).astype(bf)
    return [
        {"x": xh[b], "wm": wm, "wv": wv} for b in range(B)
    ]


def kernel(x, Wq, Wk, Wv, **_ignored):
    nc = _build()
    in_maps = _in_maps(x, Wq, Wk, Wv)
    res = run_bass_kernel_spmd(
        nc, in_maps, core_ids=list(range(N_CORES)), trace=False
    )
    return np.stack([res.results[b]["out"] for b in range(B)], axis=0)
